# revision 16
# baseline (speedup 1.0000x reference)
"""Multi-head attention (B=2, L=2048, D=1024, H=16, RoPE, softmax, out-proj)
on 8 Trainium2 NeuronCores.

Sharding: 2-way data parallel on batch x 4-way tensor parallel on heads.
Core c handles batch c//4 and heads 4*(c%4) .. 4*(c%4)+3. Each core:
  - projects its batch's activations with its head-slice of W_qkv
    (dk-outer accumulation so matmuls start as soon as x columns arrive)
  - applies RoPE, computes S^T = K_rope Q_rope^T per head in transposed
    layout (softmax normalizer and P.T@V then need no transposes of P);
    the K^T stationary operand is zero-padded to K=128 so the other head's
    lanes contribute exactly zero while the PE activity monitor sees a
    full-width matmul (K=64 matmuls leave the clock gated at half rate)
  - exp (no max subtraction: logits ~ N(0,1), |S| < ~6, exp is safe in fp32)
  - o~[q, 65] = P^T.T @ [V | 1] accumulated over all k in PSUM
    (col 64 = softmax denominator); normalized straight out of PSUM
  - q-half-outer loop: the first half's out-proj and ReduceScatter overlap
    the second half's attention
Host reassembles the full [2, 2048, 1024] output.

All matmuls bf16 with fp32 PSUM accumulation; softmax in fp32 (PSUM) with
bf16 P storage.
"""

import numpy as np
import ml_dtypes
from contextlib import ExitStack

import concourse.bass as bass
import concourse.tile as tile
from concourse import bacc, mybir
from concourse.bass_utils import run_bass_kernel_spmd
from concourse.masks import make_identity

BF16 = mybir.dt.bfloat16
F32 = mybir.dt.float32

B, L, D = 2, 2048, 1024
H_TOT, H = 16, 4          # total heads, heads per core
HD, HF = 64, 32           # head dim, rope freqs
DL = H * HD               # local head dims per core = 256
P = 128
KT = L // P               # 16 k-tiles
DK = D // P               # 8 contraction tiles over model dim
NCHUNK = 512
NQC = L // NCHUNK         # 4 sequence chunks (collective granularity)
QH = L // 2               # q half
ROPE_BASE = 10000.0
GROUPS = [[0, 1, 2, 3], [4, 5, 6, 7]]

_CACHED_NC = None


def _build_program():
    nc = bacc.Bacc("TRN2", target_bir_lowering=False, debug=False, num_devices=8)

    xT_ext = nc.dram_tensor("xT", [DK, 2, P, 2 * NCHUNK], BF16, kind="ExternalInput")
    wqk_ext = nc.dram_tensor("wqkT", [DK, P, 4 * P], BF16, kind="ExternalInput")
    wv_ext = nc.dram_tensor("wvT", [DK, P, DL], BF16, kind="ExternalInput")
    wo_ext = nc.dram_tensor("woT", [2, P, D], BF16, kind="ExternalInput")
    cos_ext = nc.dram_tensor("cosF", [P, L], F32, kind="ExternalInput")
    sin_ext = nc.dram_tensor("sinF", [P, L], F32, kind="ExternalInput")
    out_ext = nc.dram_tensor("out", [DL, L], F32, kind="ExternalOutput")

    partials = [nc.dram_tensor(f"partialT{c}", [D, QH], BF16) for c in range(2)]
    scats = [nc.dram_tensor(f"scatT{c}", [DL, QH], BF16) for c in range(2)]

    with tile.TileContext(nc) as tc:
        with ExitStack() as ctx:
            pers = ctx.enter_context(tc.tile_pool(name="pers", bufs=1))

            wv = pers.tile([P, DK, DL], BF16, tag="wv")
            wo = pers.tile([P, 2, D], BF16, tag="wo")
            qt = [pers.tile([P, 2, QH], BF16, tag=f"qt{i}", name=f"qt{i}")
                  for i in range(2)]                       # head-contig Q^T, per L-half
            ktz = [pers.tile([P, H, QH], BF16, tag=f"ktz{i}", name=f"ktz{i}")
                   for i in range(2)]                      # zero-padded K^T, per L-half
            v1 = pers.tile([P, KT, H * (HD + 1)], BF16, tag="v1")  # [V | 1]
            ident = pers.tile([P, P], BF16, tag="ident")

            xp = ctx.enter_context(tc.tile_pool(name="xp", bufs=1))
            xt = [[None, None] for _ in range(DK)]
            for dk in range(DK):
                for cp in range(2):
                    xt[dk][cp] = xp.tile([P, 2 * NCHUNK], BF16, tag=f"xt{dk}_{cp}",
                                         name=f"x_t{dk}_{cp}")

            # ---------------- QK projection + rope ----------------
            with ExitStack() as pctx:
                pj = pctx.enter_context(tc.tile_pool(name="proj", bufs=1))
                tmp = pctx.enter_context(tc.tile_pool(name="ptmp", bufs=4))
                pp = pctx.enter_context(tc.tile_pool(name="pjps", bufs=1, space="PSUM"))

                wqk = [pj.tile([P, 4 * P], BF16, tag=f"wqk{dk}", name=f"wqk{dk}")
                       for dk in range(DK)]
                cosf = pj.tile([P, L], F32, tag="cosf")
                sinf = pj.tile([P, L], F32, tag="sinf")
                qkr = [pj.tile([P, 4, QH], BF16, tag=f"qkr{i}", name=f"qkr{i}")
                       for i in range(2)]  # qr1 qr2 kr1 kr2, per L-half

                # load order = need order: qk weights + first x half, then
                # rope tables, then the rest
                for dk in range(DK):
                    nc.sync.dma_start(out=wqk[dk][:], in_=wqk_ext[dk])
                for dk in range(DK):
                    nc.sync.dma_start(out=xt[dk][0][:], in_=xT_ext[dk, 0])
                nc.sync.dma_start(out=cosf[:], in_=cos_ext[:])
                nc.sync.dma_start(out=sinf[:], in_=sin_ext[:])
                nc.vector.memset(ktz[0][:], 0.0)
                nc.vector.memset(ktz[1][:], 0.0)
                for dk in range(DK):
                    nc.sync.dma_start(out=xt[dk][1][:], in_=xT_ext[dk, 1])
                for dk in range(DK):
                    nc.sync.dma_start(out=wv[:, dk, :], in_=wv_ext[dk])
                for t in range(2):
                    nc.sync.dma_start(out=wo[:, t, :], in_=wo_ext[t])
                make_identity(nc, ident[:])

                # m: 0=qx1 1=qx2 2=kx1 3=kx2. Chains run one at a time
                # (m-outer) so early chains finish early: rope starts sooner
                # and the PSUM slots recycle for the next chunk pair instead
                # of all eight chains blocking on the rope reads at once.
                for cp in range(2):
                    pq = {(c, m): pp.tile([P, NCHUNK], F32, tag=f"qk{c}{m}",
                                          name=f"pq_{cp}_{c}_{m}")
                          for c in range(2) for m in range(4)}
                    for c in range(2):
                        for m in range(4):
                            for dk in range(DK):
                                nc.tensor.matmul(
                                    pq[(c, m)][:],
                                    wqk[dk][:, m * P:(m + 1) * P],
                                    xt[dk][cp][:, c * NCHUNK:(c + 1) * NCHUNK],
                                    start=(dk == 0), stop=(dk == DK - 1))
                    for c in range(2):
                        xs = slice((2 * cp + c) * NCHUNK, (2 * cp + c + 1) * NCHUNK)
                        ws = slice(c * NCHUNK, (c + 1) * NCHUNK)
                        for base in (0, 2):
                            x1, x2 = pq[(c, base)], pq[(c, base + 1)]
                            t1 = tmp.tile([P, NCHUNK], F32, tag="t1")
                            t2 = tmp.tile([P, NCHUNK], F32, tag="t2")
                            nc.vector.tensor_mul(t1[:], x1[:], cosf[:, xs])
                            nc.vector.tensor_mul(t2[:], x2[:], sinf[:, xs])
                            nc.vector.tensor_sub(qkr[cp][:, base, ws], t1[:], t2[:])
                            t3 = tmp.tile([P, NCHUNK], F32, tag="t1")
                            t4 = tmp.tile([P, NCHUNK], F32, tag="t2")
                            nc.vector.tensor_mul(t3[:], x1[:], sinf[:, xs])
                            nc.vector.tensor_mul(t4[:], x2[:], cosf[:, xs])
                            nc.vector.tensor_add(qkr[cp][:, base + 1, ws], t3[:], t4[:])

                # head-contiguous Q^T (both heads per tile) and zero-padded K^T;
                # split per L-half so attention starts after the first chunk pair
                for lh in range(2):
                    for h in range(H):
                        t, pb = h // 2, 64 * (h % 2)
                        hs = slice(32 * h, 32 * h + 32)
                        nc.sync.dma_start(out=qt[lh][pb:pb + 32, t, :], in_=qkr[lh][hs, 0, :])
                        nc.sync.dma_start(out=qt[lh][pb + 32:pb + 64, t, :], in_=qkr[lh][hs, 1, :])
                        nc.sync.dma_start(out=ktz[lh][pb:pb + 32, h, :], in_=qkr[lh][hs, 2, :])
                        nc.sync.dma_start(out=ktz[lh][pb + 32:pb + 64, h, :], in_=qkr[lh][hs, 3, :])

            # ---------------- attention + finish, q-half-outer ----------------
            with ExitStack() as actx:
                ptp = actx.enter_context(tc.tile_pool(name="ptp", bufs=1))
                fin = actx.enter_context(tc.tile_pool(name="fin", bufs=1))
                aps = actx.enter_context(tc.tile_pool(name="aps", bufs=1, space="PSUM"))

                first_v = True
                pre_pts = {}

                def st_exp(qh_, h_, k_, pt2_, ki_):
                    st = aps.tile([P, QH], F32, tag="st", bufs=2,
                                  name=f"st_{qh_}_{h_}_{k_}")
                    for qc in range(2):
                        cs = slice(qc * NCHUNK, (qc + 1) * NCHUNK)
                        nc.tensor.matmul(
                            st[:, cs],
                            ktz[k_ // 8][:, h_, (k_ % 8) * P:(k_ % 8 + 1) * P],
                            qt[qh_][:, h_ // 2, cs], start=True, stop=True)
                    nc.scalar.activation(pt2_[:, ki_, :], st[:],
                                         mybir.ActivationFunctionType.Exp)

                for qh in range(2):
                    qhs = slice(qh * QH, (qh + 1) * QH)
                    o_nrm = fin.tile([P, QH // P, DL], BF16, tag="onrm", bufs=2)
                    onT = fin.tile([P, 2, QH], BF16, tag="onT", bufs=2)
                    for h in range(H):
                        t = h // 2
                        vs = slice(h * (HD + 1), (h + 1) * (HD + 1))
                        pts = []
                        for k2 in range(KT // 2):
                            if (qh, h, k2) in pre_pts:
                                pts.append(pre_pts.pop((qh, h, k2)))
                                continue
                            pt2 = ptp.tile([P, 2, QH], BF16, tag="pt", bufs=12)
                            pts.append(pt2)
                            for ki in range(2):
                                st_exp(qh, h, 2 * k2 + ki, pt2, ki)

                        if first_v:
                            # V projection, emitted here so it fills the PE
                            # while the first head's exps run
                            first_v = False
                            for k in range(KT):
                                pv = aps.tile([P, DL], F32, tag="misc", bufs=2)
                                for dk in range(DK):
                                    nc.tensor.matmul(
                                        pv[:],
                                        xt[dk][k // 8][:, (k % 8) * P:(k % 8 + 1) * P],
                                        wv[:, dk, :],
                                        start=(dk == 0), stop=(dk == DK - 1))
                                src3 = pv[:].rearrange("p (h d) -> p h d", h=H)
                                dst3 = v1[:, k, :].rearrange("p (h d) -> p h d", h=H)
                                nc.vector.tensor_copy(dst3[:, :, 0:HD], src3)
                                nc.vector.memset(dst3[:, :, HD:HD + 1], 1.0)

                        # Pre-emit the next head's first two S^T+exp
                        # k-tiles BEFORE this head's PV chains: the PE stream
                        # is in-order, so otherwise the scalar engine idles
                        # while the PE drains the PV tail at every boundary.
                        nqh, nh = (qh, h + 1) if h + 1 < H else (qh + 1, 0)
                        if nqh < 2:
                            ptn = ptp.tile([P, 2, QH], BF16, tag="pt", bufs=12,
                                           name=f"ptn_{nqh}_{nh}")
                            pre_pts[(nqh, nh, 0)] = ptn
                            for ki in range(2):
                                st_exp(nqh, nh, ki, ptn, ki)

                        # PV: full-k accumulation chains, one per q-tile
                        for q in range(QH // P):
                            ob = aps.tile([P, HD + 1], F32, tag="ob", bufs=2)
                            for k in range(KT):
                                nc.tensor.matmul(
                                    ob[:], pts[k // 2][:, k % 2, q * P:(q + 1) * P],
                                    v1[:, k, vs],
                                    start=(k == 0), stop=(k == KT - 1))
                            rec = fin.tile([P, 1], F32, tag="rec", bufs=4)
                            nc.vector.reciprocal(rec[:], ob[:, HD:HD + 1])
                            nc.vector.tensor_scalar(
                                out=o_nrm[:, q, h * HD:(h + 1) * HD],
                                in0=ob[:, 0:HD],
                                scalar1=rec[:], scalar2=None,
                                op0=mybir.AluOpType.mult)
                            if h == H - 1:
                                # last head: this q-tile of o_nrm is complete,
                                # transpose it for the out-projection now
                                for t in range(2):
                                    ptr = aps.tile([P, P], BF16, tag="misc", bufs=2)
                                    nc.tensor.transpose(
                                        ptr[:], o_nrm[:, q, t * P:(t + 1) * P],
                                        ident[:])
                                    nc.vector.tensor_copy(
                                        onT[:, t, q * P:(q + 1) * P], ptr[:])

                    for qcw in range(2):
                        ws = slice(qcw * NCHUNK, (qcw + 1) * NCHUNK)
                        for ot in range(DK):
                            # half 0's out-proj runs under half 1's attention,
                            # so it must not touch the st slots; half 1's runs
                            # after the last exp and can reuse them
                            po = aps.tile([P, NCHUNK], F32,
                                          tag="misc" if qh == 0 else "st", bufs=2)
                            for t in range(2):
                                nc.tensor.matmul(
                                    po[:], wo[:, t, ot * P:(ot + 1) * P], onT[:, t, ws],
                                    start=(t == 0), stop=(t == 1))
                            so = fin.tile([P, NCHUNK], BF16, tag="so", bufs=4)
                            nc.vector.tensor_copy(so[:], po[:])
                            nc.sync.dma_start(
                                out=partials[qh][ot * P:(ot + 1) * P, ws], in_=so[:])
                        del po, so
                    nc.gpsimd.collective_compute(
                        "ReduceScatter", mybir.AluOpType.add,
                        replica_groups=GROUPS,
                        ins=[partials[qh][:]], outs=[scats[qh][:]])
                    nc.gpsimd.dma_start(out=out_ext[:, qhs], in_=scats[qh][:])

    nc.compile()
    return nc


def _prep_inputs(x, W_qkv, W_out):
    """Host-side sharding / layout prep -> per-core input maps."""
    Wq, Wk, Wv = W_qkv[0:D], W_qkv[D:2 * D], W_qkv[2 * D:3 * D]
    inv = 1.0 / (ROPE_BASE ** (np.arange(0, HD, 2, dtype=np.float64) / HD))
    pos = np.arange(L, dtype=np.float64)
    ang = pos[:, None] * inv[None, :]                     # [L, 32]
    cosF = np.tile(np.cos(ang).T, (H, 1)).astype(np.float32)  # [128, L]
    sinF = np.tile(np.sin(ang).T, (H, 1)).astype(np.float32)

    scale = float(HD) ** -0.5
    in_maps = []
    for c in range(8):
        b, g = c // 4, c % 4
        rows_x1 = np.array([64 * (4 * g + h) + 2 * f for h in range(H) for f in range(HF)])
        rows_x2 = rows_x1 + 1
        wqkT = np.concatenate([
            (scale * Wq[rows_x1]).T, (scale * Wq[rows_x2]).T,
            Wk[rows_x1].T, Wk[rows_x2].T], axis=1)        # [1024, 512]
        wvT = Wv[DL * g:DL * (g + 1)].T                   # [1024, 256]
        woT = W_out[:, DL * g:DL * (g + 1)].T             # [256, 1024]
        xTt = x[b].T.reshape(DK, P, 2, 2 * NCHUNK).transpose(0, 2, 1, 3)
        in_maps.append({
            "xT": np.ascontiguousarray(xTt).astype(ml_dtypes.bfloat16),
            "wqkT": np.ascontiguousarray(wqkT.reshape(DK, P, 4 * P)).astype(ml_dtypes.bfloat16),
            "wvT": np.ascontiguousarray(wvT.reshape(DK, P, DL)).astype(ml_dtypes.bfloat16),
            "woT": np.ascontiguousarray(woT.reshape(2, P, D)).astype(ml_dtypes.bfloat16),
            "cosF": cosF, "sinF": sinF,
        })
    return in_maps


def _run(in_maps, trace=False):
    global _CACHED_NC
    if _CACHED_NC is None:
        _CACHED_NC = _build_program()
    kw = dict(trace=True) if trace else {}
    return run_bass_kernel_spmd(_CACHED_NC, in_maps, list(range(8)), **kw)


def kernel(x, W_qkv, W_out, _trace=False):
    x = np.asarray(x, dtype=np.float32)
    W_qkv = np.asarray(W_qkv, dtype=np.float32)
    W_out = np.asarray(W_out, dtype=np.float32)
    res = _run(_prep_inputs(x, W_qkv, W_out), trace=_trace)
    out = np.empty((B, L, D), dtype=np.float32)
    for b in range(B):
        outT = np.concatenate([res.results[4 * b + j]["out"] for j in range(4)], axis=0)
        out[b] = outT.T
    if _trace:
        kernel.last_exec_time_ns = res.exec_time_ns
        kernel.last_trace = res.instructions_and_trace
    return out


# revision 17
# speedup vs baseline: 1.0529x; 1.0529x over previous
"""Multi-head attention (B=2, L=2048, D=1024, H=16, RoPE, softmax, out-proj)
on 8 Trainium2 NeuronCores.

Sharding: 2-way data parallel on batch x 4-way tensor parallel on heads.
Core c handles batch c//4 and heads 4*(c%4) .. 4*(c%4)+3. Each core:
  - projects its batch's activations with its head-slice of W_qkv
    (dk-outer accumulation so matmuls start as soon as x columns arrive)
  - applies RoPE, computes S^T = K_rope Q_rope^T per head in transposed
    layout (softmax normalizer and P.T@V then need no transposes of P);
    the K^T stationary operand is zero-padded to K=128 so the other head's
    lanes contribute exactly zero while the PE activity monitor sees a
    full-width matmul (K=64 matmuls leave the clock gated at half rate)
  - exp (no max subtraction: logits ~ N(0,1), |S| < ~6, exp is safe in fp32)
  - o~[q, 65] = P^T.T @ [V | 1] accumulated over all k in PSUM
    (col 64 = softmax denominator); normalized straight out of PSUM
  - q-half-outer loop: the first half's out-proj and ReduceScatter overlap
    the second half's attention
Host reassembles the full [2, 2048, 1024] output.

All matmuls bf16 with fp32 PSUM accumulation; softmax in fp32 (PSUM) with
bf16 P storage.
"""

import numpy as np
import ml_dtypes
from contextlib import ExitStack

import concourse.bass as bass
import concourse.tile as tile
from concourse import bacc, mybir
from concourse.bass_utils import run_bass_kernel_spmd
from concourse.masks import make_identity

BF16 = mybir.dt.bfloat16
F32 = mybir.dt.float32

B, L, D = 2, 2048, 1024
H_TOT, H = 16, 4          # total heads, heads per core
HD, HF = 64, 32           # head dim, rope freqs
DL = H * HD               # local head dims per core = 256
P = 128
KT = L // P               # 16 k-tiles
DK = D // P               # 8 contraction tiles over model dim
NCHUNK = 512
NQC = L // NCHUNK         # 4 sequence chunks (collective granularity)
QH = L // 2               # q half
ROPE_BASE = 10000.0
GROUPS = [[0, 1, 2, 3], [4, 5, 6, 7]]

_CACHED_NC = None


def _build_program():
    nc = bacc.Bacc("TRN2", target_bir_lowering=False, debug=False, num_devices=8)

    xT_ext = nc.dram_tensor("xT", [DK, 2, P, 2 * NCHUNK], BF16, kind="ExternalInput")
    wqk_ext = nc.dram_tensor("wqkT", [DK, P, 4 * P], BF16, kind="ExternalInput")
    wv_ext = nc.dram_tensor("wvT", [DK, P, DL], BF16, kind="ExternalInput")
    wo_ext = nc.dram_tensor("woT", [2, P, D], BF16, kind="ExternalInput")
    cos_ext = nc.dram_tensor("cosF", [P, L], F32, kind="ExternalInput")
    sin_ext = nc.dram_tensor("sinF", [P, L], F32, kind="ExternalInput")
    out_ext = nc.dram_tensor("out", [DL, L], F32, kind="ExternalOutput")

    partials = [nc.dram_tensor(f"partialT{c}", [D, QH], BF16) for c in range(2)]
    scats = [nc.dram_tensor(f"scatT{c}", [DL, QH], BF16) for c in range(2)]

    with tile.TileContext(nc) as tc:
        with ExitStack() as ctx:
            pers = ctx.enter_context(tc.tile_pool(name="pers", bufs=1))

            wv = pers.tile([P, DK, DL], BF16, tag="wv")
            wo = pers.tile([P, 2, D], BF16, tag="wo")
            qt = [pers.tile([P, 2, QH], BF16, tag=f"qt{i}", name=f"qt{i}")
                  for i in range(2)]                       # head-contig Q^T, per L-half
            ktz = [pers.tile([P, H, QH], BF16, tag=f"ktz{i}", name=f"ktz{i}")
                   for i in range(2)]                      # zero-padded K^T, per L-half
            v1 = pers.tile([P, KT, H * (HD + 1)], BF16, tag="v1")  # [V | 1]
            ident = pers.tile([P, P], BF16, tag="ident")

            xp = ctx.enter_context(tc.tile_pool(name="xp", bufs=1))
            xt = [[None, None] for _ in range(DK)]
            for dk in range(DK):
                for cp in range(2):
                    xt[dk][cp] = xp.tile([P, 2 * NCHUNK], BF16, tag=f"xt{dk}_{cp}",
                                         name=f"x_t{dk}_{cp}")

            # ---------------- QK projection + rope ----------------
            with ExitStack() as pctx:
                pj = pctx.enter_context(tc.tile_pool(name="proj", bufs=1))
                tmp = pctx.enter_context(tc.tile_pool(name="ptmp", bufs=4))
                pp = pctx.enter_context(tc.tile_pool(name="pjps", bufs=1, space="PSUM"))

                wqk = [pj.tile([P, 4 * P], BF16, tag=f"wqk{dk}", name=f"wqk{dk}")
                       for dk in range(DK)]
                cosf = pj.tile([P, L], F32, tag="cosf")
                sinf = pj.tile([P, L], F32, tag="sinf")
                qkr = [pj.tile([P, 4, QH], BF16, tag=f"qkr{i}", name=f"qkr{i}")
                       for i in range(2)]  # qr1 qr2 kr1 kr2, per L-half

                # load order = need order: qk weights + first x half, then
                # rope tables, then the rest
                for dk in range(DK):
                    nc.sync.dma_start(out=wqk[dk][:], in_=wqk_ext[dk])
                for dk in range(DK):
                    nc.sync.dma_start(out=xt[dk][0][:], in_=xT_ext[dk, 0])
                nc.sync.dma_start(out=cosf[:], in_=cos_ext[:])
                nc.sync.dma_start(out=sinf[:], in_=sin_ext[:])
                nc.vector.memset(ktz[0][:], 0.0)
                nc.vector.memset(ktz[1][:], 0.0)
                for dk in range(DK):
                    nc.sync.dma_start(out=xt[dk][1][:], in_=xT_ext[dk, 1])
                for dk in range(DK):
                    nc.sync.dma_start(out=wv[:, dk, :], in_=wv_ext[dk])
                for t in range(2):
                    nc.sync.dma_start(out=wo[:, t, :], in_=wo_ext[t])
                make_identity(nc, ident[:])

                # m: 0=qx1 1=qx2 2=kx1 3=kx2. Chains run one at a time
                # (m-outer) so early chains finish early: rope starts sooner
                # and the PSUM slots recycle for the next chunk pair instead
                # of all eight chains blocking on the rope reads at once.
                for cp in range(2):
                    pq = {(c, m): pp.tile([P, NCHUNK], F32, tag=f"qk{c}{m}",
                                          name=f"pq_{cp}_{c}_{m}")
                          for c in range(2) for m in range(4)}
                    for c in range(2):
                        for m in range(4):
                            for dk in range(DK):
                                nc.tensor.matmul(
                                    pq[(c, m)][:],
                                    wqk[dk][:, m * P:(m + 1) * P],
                                    xt[dk][cp][:, c * NCHUNK:(c + 1) * NCHUNK],
                                    start=(dk == 0), stop=(dk == DK - 1))
                    for c in range(2):
                        xs = slice((2 * cp + c) * NCHUNK, (2 * cp + c + 1) * NCHUNK)
                        ws = slice(c * NCHUNK, (c + 1) * NCHUNK)
                        for base in (0, 2):
                            x1, x2 = pq[(c, base)], pq[(c, base + 1)]
                            t1 = tmp.tile([P, NCHUNK], F32, tag="t1")
                            t2 = tmp.tile([P, NCHUNK], F32, tag="t2")
                            nc.vector.tensor_mul(t1[:], x1[:], cosf[:, xs])
                            nc.vector.tensor_mul(t2[:], x2[:], sinf[:, xs])
                            nc.vector.tensor_sub(qkr[cp][:, base, ws], t1[:], t2[:])
                            t3 = tmp.tile([P, NCHUNK], F32, tag="t1")
                            t4 = tmp.tile([P, NCHUNK], F32, tag="t2")
                            nc.vector.tensor_mul(t3[:], x1[:], sinf[:, xs])
                            nc.vector.tensor_mul(t4[:], x2[:], cosf[:, xs])
                            nc.vector.tensor_add(qkr[cp][:, base + 1, ws], t3[:], t4[:])

                # head-contiguous Q^T (both heads per tile) and zero-padded K^T;
                # split per L-half so attention starts after the first chunk pair
                for lh in range(2):
                    for h in range(H):
                        t, pb = h // 2, 64 * (h % 2)
                        hs = slice(32 * h, 32 * h + 32)
                        nc.sync.dma_start(out=qt[lh][pb:pb + 32, t, :], in_=qkr[lh][hs, 0, :])
                        nc.sync.dma_start(out=qt[lh][pb + 32:pb + 64, t, :], in_=qkr[lh][hs, 1, :])
                        nc.sync.dma_start(out=ktz[lh][pb:pb + 32, h, :], in_=qkr[lh][hs, 2, :])
                        nc.sync.dma_start(out=ktz[lh][pb + 32:pb + 64, h, :], in_=qkr[lh][hs, 3, :])

            # ---------------- attention + finish, q-half-outer ----------------
            with ExitStack() as actx:
                ptp = actx.enter_context(tc.tile_pool(name="ptp", bufs=1))
                fin = actx.enter_context(tc.tile_pool(name="fin", bufs=1))
                aps = actx.enter_context(tc.tile_pool(name="aps", bufs=1, space="PSUM"))

                first_v = True
                for qh in range(2):
                    qhs = slice(qh * QH, (qh + 1) * QH)
                    o_nrm = fin.tile([P, QH // P, DL], BF16, tag="onrm", bufs=2)
                    onT = fin.tile([P, 2, QH], BF16, tag="onT", bufs=2)
                    for h in range(H):
                        t = h // 2
                        vs = slice(h * (HD + 1), (h + 1) * (HD + 1))
                        pts = []
                        for k2 in range(KT // 2):
                            pt2 = ptp.tile([P, 2, QH], BF16, tag="pt", bufs=12)
                            pts.append(pt2)
                            for ki in range(2):
                                k = 2 * k2 + ki
                                ks = slice(k * P, (k + 1) * P)
                                st = aps.tile([P, QH], F32, tag="st", bufs=2)
                                for qc in range(2):
                                    cs = slice(qc * NCHUNK, (qc + 1) * NCHUNK)
                                    nc.tensor.matmul(
                                        st[:, cs], ktz[k // 8][:, h, (k % 8) * P:(k % 8 + 1) * P],
                                        qt[qh][:, t, cs],
                                        start=True, stop=True)
                                nc.scalar.activation(
                                    pt2[:, ki, :], st[:],
                                    mybir.ActivationFunctionType.Exp)

                        if first_v:
                            # V projection, emitted here so it fills the PE
                            # while the first head's exps run
                            first_v = False
                            for k in range(KT):
                                pv = aps.tile([P, DL], F32, tag="misc", bufs=2)
                                for dk in range(DK):
                                    nc.tensor.matmul(
                                        pv[:],
                                        xt[dk][k // 8][:, (k % 8) * P:(k % 8 + 1) * P],
                                        wv[:, dk, :],
                                        start=(dk == 0), stop=(dk == DK - 1))
                                src3 = pv[:].rearrange("p (h d) -> p h d", h=H)
                                dst3 = v1[:, k, :].rearrange("p (h d) -> p h d", h=H)
                                nc.vector.tensor_copy(dst3[:, :, 0:HD], src3)
                                nc.vector.memset(dst3[:, :, HD:HD + 1], 1.0)

                        # PV: full-k accumulation chains, one per q-tile
                        for q in range(QH // P):
                            ob = aps.tile([P, HD + 1], F32, tag="ob", bufs=2)
                            for k in range(KT):
                                nc.tensor.matmul(
                                    ob[:], pts[k // 2][:, k % 2, q * P:(q + 1) * P],
                                    v1[:, k, vs],
                                    start=(k == 0), stop=(k == KT - 1))
                            rec = fin.tile([P, 1], F32, tag="rec", bufs=4)
                            nc.vector.reciprocal(rec[:], ob[:, HD:HD + 1])
                            nc.vector.tensor_scalar(
                                out=o_nrm[:, q, h * HD:(h + 1) * HD],
                                in0=ob[:, 0:HD],
                                scalar1=rec[:], scalar2=None,
                                op0=mybir.AluOpType.mult)
                            if h == H - 1:
                                # last head: this q-tile of o_nrm is complete,
                                # transpose it for the out-projection now
                                for t in range(2):
                                    ptr = aps.tile([P, P], BF16, tag="misc", bufs=2)
                                    nc.tensor.transpose(
                                        ptr[:], o_nrm[:, q, t * P:(t + 1) * P],
                                        ident[:])
                                    nc.vector.tensor_copy(
                                        onT[:, t, q * P:(q + 1) * P], ptr[:])

                    for qcw in range(2):
                        ws = slice(qcw * NCHUNK, (qcw + 1) * NCHUNK)
                        for ot in range(DK):
                            # half 0's out-proj runs under half 1's attention,
                            # so it must not touch the st slots; half 1's runs
                            # after the last exp and can reuse them
                            po = aps.tile([P, NCHUNK], F32,
                                          tag="misc" if qh == 0 else "st", bufs=2)
                            for t in range(2):
                                nc.tensor.matmul(
                                    po[:], wo[:, t, ot * P:(ot + 1) * P], onT[:, t, ws],
                                    start=(t == 0), stop=(t == 1))
                            so = fin.tile([P, NCHUNK], BF16, tag="so", bufs=4)
                            nc.vector.tensor_copy(so[:], po[:])
                            nc.sync.dma_start(
                                out=partials[qh][ot * P:(ot + 1) * P, ws], in_=so[:])
                        del po, so
                    nc.gpsimd.collective_compute(
                        "ReduceScatter", mybir.AluOpType.add,
                        replica_groups=GROUPS,
                        ins=[partials[qh][:]], outs=[scats[qh][:]])
                    nc.gpsimd.dma_start(out=out_ext[:, qhs], in_=scats[qh][:])

    nc.compile()
    return nc


def _prep_inputs(x, W_qkv, W_out):
    """Host-side sharding / layout prep -> per-core input maps."""
    Wq, Wk, Wv = W_qkv[0:D], W_qkv[D:2 * D], W_qkv[2 * D:3 * D]
    inv = 1.0 / (ROPE_BASE ** (np.arange(0, HD, 2, dtype=np.float64) / HD))
    pos = np.arange(L, dtype=np.float64)
    ang = pos[:, None] * inv[None, :]                     # [L, 32]
    cosF = np.tile(np.cos(ang).T, (H, 1)).astype(np.float32)  # [128, L]
    sinF = np.tile(np.sin(ang).T, (H, 1)).astype(np.float32)

    scale = float(HD) ** -0.5
    in_maps = []
    for c in range(8):
        b, g = c // 4, c % 4
        rows_x1 = np.array([64 * (4 * g + h) + 2 * f for h in range(H) for f in range(HF)])
        rows_x2 = rows_x1 + 1
        wqkT = np.concatenate([
            (scale * Wq[rows_x1]).T, (scale * Wq[rows_x2]).T,
            Wk[rows_x1].T, Wk[rows_x2].T], axis=1)        # [1024, 512]
        wvT = Wv[DL * g:DL * (g + 1)].T                   # [1024, 256]
        woT = W_out[:, DL * g:DL * (g + 1)].T             # [256, 1024]
        xTt = x[b].T.reshape(DK, P, 2, 2 * NCHUNK).transpose(0, 2, 1, 3)
        in_maps.append({
            "xT": np.ascontiguousarray(xTt).astype(ml_dtypes.bfloat16),
            "wqkT": np.ascontiguousarray(wqkT.reshape(DK, P, 4 * P)).astype(ml_dtypes.bfloat16),
            "wvT": np.ascontiguousarray(wvT.reshape(DK, P, DL)).astype(ml_dtypes.bfloat16),
            "woT": np.ascontiguousarray(woT.reshape(2, P, D)).astype(ml_dtypes.bfloat16),
            "cosF": cosF, "sinF": sinF,
        })
    return in_maps


def _run(in_maps, trace=False):
    global _CACHED_NC
    if _CACHED_NC is None:
        _CACHED_NC = _build_program()
    kw = dict(trace=True) if trace else {}
    return run_bass_kernel_spmd(_CACHED_NC, in_maps, list(range(8)), **kw)


def kernel(x, W_qkv, W_out, _trace=False):
    x = np.asarray(x, dtype=np.float32)
    W_qkv = np.asarray(W_qkv, dtype=np.float32)
    W_out = np.asarray(W_out, dtype=np.float32)
    res = _run(_prep_inputs(x, W_qkv, W_out), trace=_trace)
    out = np.empty((B, L, D), dtype=np.float32)
    for b in range(B):
        outT = np.concatenate([res.results[4 * b + j]["out"] for j in range(4)], axis=0)
        out[b] = outT.T
    if _trace:
        kernel.last_exec_time_ns = res.exec_time_ns
        kernel.last_trace = res.instructions_and_trace
    return out
